# revision 33
# baseline (speedup 1.0000x reference)
"""ConvTranspose3d(64->32, k=3, stride=2, pad=1, out_pad=1, dilation=2) on 8 NeuronCores.

Math: with stride=2, dilation=2, padding=1, k=3, output position o = 2i + 2k - 1
is odd in every spatial dim, so the transposed conv collapses to a dense 3^3
conv y = conv3d(x, wc, padding=1) on the 32^3 grid (wc = flip(transpose(w))),
scattered into the odd sub-lattice of the 66^3 output; every other output
voxel is just bias (host fills those).

Sharding: 8 shards = 2 batches x 4 depth-blocks of 8 conv planes. Implicit
GEMM per core: M = (c_out=32 x 4 depth planes) on PSUM partitions, K =
(64 c_in x 2 input planes) via a block-Toeplitz stationary operand (bf16),
N = 512 hw pixels per matmul, 27 matmuls (9 hw taps x 3 K-chunks) per PSUM
bank. 27 passes/group is row-optimal: every K-row (plane, ci) is needed by
some output, and a pass is N-cycle-bound (~216ns) regardless of dtype/K.

Device writes only the 32^3 conv lattice (bf16, [128, 4, 16, 32] per core);
host broadcasts bias into the even sub-lattice. Warmup matmuls off the
const pool ride out the PE p-state ramp during the initial DMA wait.
"""

import sys

sys.path.insert(0, "/opt/trn_rl_repo")

import numpy as np
import ml_dtypes

N_CORES = 8
D_BLOCKS = 4  # depth blocks per batch
G_PER_CORE = 8  # conv output planes per core

_cache = {}


def _build_nc():
    import concourse.bass as bass
    import concourse.tile as tile
    from concourse import bacc, mybir

    dt = mybir.dt
    nc = bacc.Bacc("TRN2", target_bir_lowering=False, debug=False,
                   num_devices=N_CORES)

    # xs: 5 pairs of adjacent (zero-padded) input depth planes; partition
    # p = dpi*64 + ci. tcw: 27 block-Toeplitz stationary matrices, columns
    # (chunk*9 + tap)*128 + (co*4 + gb). bias128: p = co*4+j -> bias[co].
    xs = nc.dram_tensor("xs", [5, 128, 34, 34], dt.bfloat16,
                        kind="ExternalInput")
    tcw = nc.dram_tensor("tcw", [128, 27 * 128], dt.bfloat16,
                         kind="ExternalInput")
    bias = nc.dram_tensor("bias", [128, 1], dt.float32, kind="ExternalInput")
    # compact conv-lattice output: [co*4+gb, group g=2b+hh, h16, w]
    out = nc.dram_tensor("out", [128, 4, 16, 32], dt.bfloat16,
                         kind="ExternalOutput")

    with tile.TileContext(nc) as tc:
        with (
            tc.tile_pool(name="tw", bufs=1) as tw_pool,
            tc.tile_pool(name="xp", bufs=1) as xp_pool,
            tc.tile_pool(name="bias", bufs=1) as bias_pool,
            tc.tile_pool(name="og", bufs=1) as og_pool,
            tc.tile_pool(name="psw", bufs=1, space="PSUM") as psw_pool,
            tc.tile_pool(name="ps", bufs=4, space="PSUM") as ps_pool,
        ):
            tw_t = tw_pool.tile([128, 27 * 128], dt.bfloat16)
            xpt = xp_pool.tile([128, 5, 34, 34], dt.bfloat16)
            bias_t = bias_pool.tile([128, 1], dt.float32)

            # PE p-state warmup during the initial DMA wait: stream 512-col
            # matmuls off the preamble-initialized const pool (no runtime
            # deps, so these start the moment the PE queue opens)
            cone = nc.const_aps.aps[(dt.bfloat16, 1.0)]
            psw = psw_pool.tile([128, 16, 32], dt.float32)
            for _ in range(24):
                nc.tensor.matmul(psw[0:1, 0:4, :], cone,
                                 cone.broadcast_to((128, 4, 32)),
                                 start=True, stop=True)

            def ldx(plo, phi, r0, r1, eng):
                eng.dma_start(xpt[:, plo:phi, r0:r1, :],
                              xs[plo:phi, :, r0:r1, :]
                              .rearrange("p q r c -> q p r c"))

            def ldtw(lo, hi, eng):
                eng.dma_start(tw_t[:, lo * 128:hi * 128],
                              tcw[:, lo * 128:hi * 128])

            # first-use-ordered loads: tcw split across sync+gpsimd in
            # parallel (one queue can't keep pace with the matmul stream),
            # xp pairs 3-4 deferred behind the critical pieces
            ldx(0, 1, 0, 18, nc.scalar)   # pass 0 rhs
            ldtw(0, 4, nc.sync)           # passes 0-3 weights
            ldtw(4, 12, nc.gpsimd)        # SWDGE gen ~1us: give it slack
            ldx(1, 2, 0, 18, nc.scalar)   # passes 9-17 rhs (own sem)
            ldtw(12, 20, nc.sync)
            ldx(2, 3, 0, 18, nc.scalar)   # passes 18-26 rhs
            ldtw(20, 27, nc.gpsimd)
            ldx(0, 3, 18, 34, nc.scalar)  # hh=1 groups
            nc.sync.dma_start(bias_t[:], bias[:])
            ldx(3, 5, 0, 34, nc.scalar)   # b=1 groups, after critical pieces

            prev_last_mm = None
            for b in range(2):
                for hh in range(2):
                    g = 2 * b + hh
                    h0 = 16 * hh
                    ps = ps_pool.tile([128, 16, 32], dt.float32)
                    i = 0
                    for c in range(3):
                        for t9 in range(9):
                            kh, kw = t9 // 3, t9 % 3
                            lhsT = tw_t[:, (c * 9 + t9) * 128:
                                        (c * 9 + t9 + 1) * 128]
                            rhs = xpt[:, 2 * b + c,
                                      h0 + kh:h0 + kh + 16, kw:kw + 32]
                            mm = nc.tensor.matmul(ps[:], lhsT, rhs,
                                                  start=(i == 0),
                                                  stop=(i == 26))
                            # keep the PE's static order group-contiguous so
                            # each flush fires right after its 27th matmul
                            if i == 0 and prev_last_mm is not None:
                                tile.add_dep_helper(
                                    mm.ins, prev_last_mm.ins, sync=False,
                                    reason="group-contiguous PE order")
                            i += 1
                    prev_last_mm = mm
                    og = og_pool.tile([128, 16, 32], dt.bfloat16,
                                      tag=f"og{g}")
                    if g < 3:
                        nc.vector.tensor_scalar_add(og[:], ps[:], bias_t[:])
                        eng = nc.sync if g % 2 == 0 else nc.scalar
                        eng.dma_start(out[:, g], og[:])
                    else:
                        # tail: add in two pieces so the first half's flush
                        # overlaps the second half's add
                        nc.vector.tensor_scalar_add(og[:, 0:8], ps[:, 0:8],
                                                    bias_t[:])
                        nc.sync.dma_start(out[:, g, 0:8], og[:, 0:8])
                        nc.vector.tensor_scalar_add(og[:, 8:16], ps[:, 8:16],
                                                    bias_t[:])
                        nc.scalar.dma_start(out[:, g, 8:16], og[:, 8:16])

    nc.compile()
    return nc


def _prep_shared(weight, bias):
    # wc[co, ci, kd, kh, kw] = weight[ci, co, 2-kd, 2-kh, 2-kw]
    wc = np.flip(np.transpose(weight, (1, 0, 2, 3, 4)), axis=(2, 3, 4))
    # full pre-built Toeplitz: tcw[dpi*64+ci, (c*9+t)*128 + co*4 + gb]
    tcw = np.zeros((128, 27, 128), np.float32)
    for c in range(3):
        for dpi in range(2):
            for gb in range(4):
                kd = 2 * c + dpi - gb
                if 0 <= kd <= 2:
                    arr = wc[:, :, kd].reshape(32, 64, 9).transpose(1, 2, 0)
                    tcw[dpi * 64:(dpi + 1) * 64,
                        c * 9:(c + 1) * 9, gb::4] = arr
    tcw = np.ascontiguousarray(
        tcw.reshape(128, 27 * 128).astype(ml_dtypes.bfloat16))
    bias128 = np.ascontiguousarray(
        np.repeat(bias.astype(np.float32), 4).reshape(128, 1))
    return tcw, bias128


def _make_slab(x, n, cblk):
    # 5 pairs of spatially padded planes (34x34, zero border);
    # pair p = unpadded planes (8c-1+2p, 8c+2p)
    xs = np.zeros((5, 128, 34, 34), ml_dtypes.bfloat16)
    lo = G_PER_CORE * cblk - 1
    for p in range(5):
        for dpi in range(2):
            d = lo + 2 * p + dpi
            if 0 <= d < 32:
                xs[p, dpi * 64:(dpi + 1) * 64, 1:33, 1:33] = \
                    x[n, :, d].astype(ml_dtypes.bfloat16)
    return xs


def kernel(x, weight, bias):
    from concourse.bass_utils import run_bass_kernel_spmd

    if "nc" not in _cache:
        _cache["nc"] = _build_nc()
    nc = _cache["nc"]

    x = np.asarray(x, np.float32)
    weight = np.asarray(weight, np.float32)
    bias = np.asarray(bias, np.float32)

    tcw, bias128 = _prep_shared(weight, bias)
    in_maps = []
    for core in range(N_CORES):
        n, cblk = divmod(core, D_BLOCKS)
        in_maps.append({"xs": _make_slab(x, n, cblk), "tcw": tcw,
                        "bias": bias128})

    res = run_bass_kernel_spmd(nc, in_maps, core_ids=list(range(N_CORES)))

    # even sub-lattice (any even coordinate) is pure bias; conv results live
    # on the odd lattice [1:64:2]^3 of the 66^3 volume
    full = np.empty((2, 32, 66, 66, 66), np.float32)
    full[:] = bias[None, :, None, None, None]
    for core in range(N_CORES):
        n, cblk = divmod(core, D_BLOCKS)
        arr = res.results[core]["out"].astype(np.float32)  # (128,4,16,32)
        conv = (arr.reshape(32, 4, 2, 2, 16, 32)
                .transpose(0, 2, 1, 3, 4, 5)   # [co, b, gb, hh, h16, w]
                .reshape(32, 8, 32, 32))
        full[n, :, 16 * cblk + 1:16 * cblk + 17:2, 1:64:2, 1:64:2] = conv
    return full
